# revision 44
# baseline (speedup 1.0000x reference)
"""Longformer sliding-window + global attention layer on 8 Trainium2 NeuronCores.

Sharding: sequence-parallel over the 4096 tokens (512 per core, all 12 heads).
Each core recomputes the k/v halo (256 tokens each side) and the 64 global
k/v tokens locally from zero-padded hsT input, so the program is uniform SPMD.
The global-query rows (first 64 tokens attend to everything) are computed as
flash-style partial sums over each core's 512 tokens and combined with an
on-device AllReduce (hidden under the banded phase); every core finalizes the
identical 64 global rows.

Layout strategy (all matmuls bf16, accumulation fp32 PSUM):
  - hsT [hidden, tokens] feeds projections in both orientations. The 64
    global tokens appear twice at the tail (cols 1024:1088 and 1088:1152) so
    the natural-v projection materializes global v rows in both partition
    halves, letting the two heads of a pair use disjoint PE row groups.
  - Banded attention processes heads in pairs (head dims at partitions 0:64
    and 64:128): their K=64 score matmuls target disjoint PE row groups and
    run concurrently. Scores are transposed ([keys, queries]) over 8 extended
    window key tiles (jx) in pairs sharing a 2-bank PSUM tile so one exp
    covers up to 1024 columns. 0/1 mask multiplies cover only band/col
    boundary regions. PV accumulates into a [66, 512] PSUM tile via natural-v
    tiles carrying an appended ones-column, so the softmax denominator falls
    out of the same accumulation. A PE transpose + per-partition reciprocal
    scale normalizes [t, d] tiles.
  - Global-row heads compute scores directly transposed ([keys, 64]) via
    four 128-key strip matmuls per head, two heads sharing one PSUM tile and
    one 512-col exp, then PV with the exp strips as stationary operands -
    no PE transposes.
"""
import numpy as np
import ml_dtypes

import concourse.bacc as bacc
import concourse.mybir as mybir
import concourse.tile as tile
from concourse.bass_utils import run_bass_kernel_spmd

F32 = mybir.dt.float32
BF16 = mybir.dt.bfloat16
Exp = mybir.ActivationFunctionType.Exp

S, H, NH, HD = 4096, 768, 12, 64
C = 256               # chunk / one-sided window
G = 64                # global tokens
NCORE = 8
TPC = S // NCORE      # 512 tokens per core
EXT = TPC + 2 * C     # 1024 ext window
COLS = EXT + 2 * G    # 1152 = ext | glob | glob-dup
KC = H // 128         # 6 hidden chunks
VW = 66               # per-head v block: 64 v | ones | pad
SCALE = 1.0 / 8.0     # 1/sqrt(HD)

# per key-tile jx: (t0, tn) query range its chunk windows cover, and the
# t-range needing the 0/1 mask multiply
JX_T = {0: (0, 256), 1: (0, 256), 2: (0, 512), 3: (0, 512),
        4: (0, 512), 5: (0, 512), 6: (256, 512), 7: (256, 512)}
JX_MASK = {0: (0, 256), 1: (0, 256), 2: (0, 512), 3: (256, 512),
           4: (0, 256), 5: (0, 256), 6: (256, 512), 7: (256, 512)}
# pairs share one [128, 1024] PSUM tile; (2,3) first so PV start=True is full
JX_PAIRS = [(2, 3), (4, 5), (0, 1), (6, 7)]
# packed col offset of each jx's mask region in the [128, 2304] masks input
JX_PACK = {2: 0, 3: 512, 4: 768, 5: 1024, 0: 1280, 1: 1536, 6: 1792, 7: 2048}
# mask ops: (pair_index, ex-tile col range, packed range, engine)
MASK_OPS = [
    (0, (0, 512), (0, 512), "v"),         # jx2
    (0, (768, 1024), (512, 768), "v"),    # jx3
    (1, (0, 256), (768, 1024), "v"),      # jx4
    (1, (512, 768), (1024, 1280), "v"),   # jx5
    (2, (0, 512), (1280, 1792), "v"),     # jx0|jx1 contiguous
    (3, (0, 512), (1792, 2304), "v"),     # jx6|jx7 contiguous
]

_PROG_CACHE = {}


def _build_program(with_bias: bool):
    nc = bacc.Bacc("TRN2", target_bir_lowering=False, debug=False,
                   num_devices=NCORE)
    d_hsT = nc.declare_dram_parameter("hsT", [H, COLS], BF16, isOutput=False)
    d_w = {
        n: nc.declare_dram_parameter(n, [H, H], BF16, isOutput=False)
        for n in ("wq", "wk", "wv", "wkg", "wvg", "wqg")
    }
    d_masks = nc.declare_dram_parameter("masks", [128, 2304], BF16,
                                        isOutput=False)
    d_consts = nc.declare_dram_parameter("consts", [128, 128], BF16,
                                         isOutput=False)
    if with_bias:
        d_brow = nc.declare_dram_parameter("biasrow", [7, COLS], BF16,
                                           isOutput=False)
    d_out = nc.declare_dram_parameter("out", [TPC, H], BF16, isOutput=True)
    d_outg = nc.declare_dram_parameter("outg", [G, H], F32, isOutput=True)

    with tile.TileContext(nc) as tc:
        with (
            tc.tile_pool(name="const", bufs=1) as const,
            tc.tile_pool(name="wfull", bufs=2) as wfull,
            tc.tile_pool(name="work", bufs=2) as work,
            tc.tile_pool(name="w2", bufs=3) as w2,
            tc.tile_pool(name="late", bufs=1) as late,
            tc.tile_pool(name="dram", bufs=2, space="DRAM") as dram,
            tc.tile_pool(name="psQ", bufs=2, space="PSUM") as psQ,
            tc.tile_pool(name="psO", bufs=2, space="PSUM") as psO,
            tc.tile_pool(name="psT", bufs=2, space="PSUM") as psT,
        ):
            # ---- resident loads; spread the startup DMAs across queues ----
            hsb = late.tile([128, KC, COLS], BF16, tag="ph")
            wq_t = wfull.tile([128, KC, H], BF16, tag="wq")
            for kc in range(KC):
                nc.sync.dma_start(
                    out=wq_t[:, kc, :],
                    in_=d_w["wq"][128 * kc:128 * (kc + 1), :])
                eng = nc.gpsimd if kc % 2 == 0 else nc.scalar
                eng.dma_start(
                    out=hsb[:, kc, 256:768],
                    in_=d_hsT[128 * kc:128 * (kc + 1), 256:768])
            for kc in range(KC):
                eng = nc.gpsimd if kc % 2 == 0 else nc.scalar
                eng.dma_start(
                    out=hsb[:, kc, 0:256],
                    in_=d_hsT[128 * kc:128 * (kc + 1), 0:256])
                eng.dma_start(
                    out=hsb[:, kc, 768:COLS],
                    in_=d_hsT[128 * kc:128 * (kc + 1), 768:COLS])

            def load_w(name, eng=None):
                t = wfull.tile([128, KC, H], BF16, tag="wfull")
                (eng or nc.sync).dma_start(
                    out=t, in_=d_w[name].rearrange("(kc p) o -> p kc o", p=128))
                return t

            csb = const.tile([128, 128], BF16)
            nc.gpsimd.dma_start(out=csb, in_=d_consts[:])
            ident = csb
            if with_bias:
                bsb = const.tile([7, COLS], BF16)
                nc.gpsimd.dma_start(out=bsb, in_=d_brow[:])

            KCOLS = EXT + G                           # kT covers one glob copy
            kT = const.tile([128, KC, KCOLS], BF16)   # [o, t] all heads
            qT = const.tile([128, KC, TPC], BF16)
            vE = const.tile([128, 9, NH * VW], BF16)  # natural v + ones cols
            kgT = const.tile([128, KC, TPC], BF16)
            vgN = const.tile([128, 4, NH * VW], BF16)
            qgT = const.tile([128, KC, G], BF16)
            qgn = const.tile([G, H], BF16)
            msb = const.tile([128, 2304], BF16)
            nc.gpsimd.dma_start(out=msb, in_=d_masks[:])
            # ones/pad columns of the natural-v blocks (cols 64,65 of each
            # 66-block); value cols are overwritten by the projections
            nc.gpsimd.memset(
                vE.rearrange("p a (h x) -> p a h x", x=VW)[:, :, :, 64:66], 1.0)
            nc.gpsimd.memset(
                vgN.rearrange("p a (h x) -> p a h x", x=VW)[:, :, :, 64:66], 1.0)

            def proj_T(dst, wsl, segs, bias_idx, dst_off):
                # dst[o, t] = W.T @ hsT cols; wsl(kc, oc) -> [128, 128] strip
                for oc in range(KC):
                    for c0, cn in segs:
                        ps = psQ.tile([128, 512], F32, tag="psQ")
                        for kc in range(KC):
                            nc.tensor.matmul(
                                out=ps[:, 0:cn],
                                lhsT=wsl(kc, oc),
                                rhs=hsb[:, kc, c0:c0 + cn],
                                start=(kc == 0),
                                stop=(kc == KC - 1 and not with_bias),
                            )
                        if with_bias:
                            nc.tensor.matmul(
                                out=ps[:, 0:cn],
                                lhsT=bsb[1 + bias_idx:2 + bias_idx,
                                         oc * 128:(oc + 1) * 128],
                                rhs=bsb[0:1, 0:cn],
                                start=False, stop=True,
                            )
                        nc.vector.tensor_copy(
                            out=dst[:, oc, c0 - dst_off:c0 - dst_off + cn],
                            in_=ps[:, 0:cn])

            def proj_nat(dst, wsb, tts, bias_idx):
                # dst[t, head-block] with 66-stride head blocks
                for ti, tt in enumerate(tts):
                    tok0 = tt * 128
                    for o0, on in ((0, 512), (512, 256)):
                        ps = psQ.tile([128, 512], F32, tag="psQ")
                        for kc in range(KC):
                            nc.tensor.matmul(
                                out=ps[:, 0:on],
                                lhsT=hsb[:, kc, tok0:tok0 + 128],
                                rhs=wsb[:, kc, o0:o0 + on],
                                start=(kc == 0),
                                stop=(kc == KC - 1 and not with_bias),
                            )
                        if with_bias:
                            nc.tensor.matmul(
                                out=ps[:, 0:on],
                                lhsT=bsb[0:1, 0:128],
                                rhs=bsb[1 + bias_idx:2 + bias_idx, o0:o0 + on],
                                start=False, stop=True,
                            )
                        nc.vector.tensor_copy(
                            out=dst[:, ti, :].rearrange(
                                "p (h x) -> p h x", x=VW)[:, o0 // 64:(o0 + on) // 64, 0:64],
                            in_=ps[:, 0:on].rearrange("p (h x) -> p h x", x=64))

            # ---- main projections first (banded inputs ready earliest) ----
            proj_T(qT, lambda kc, oc: wq_t[:, kc, oc * 128:(oc + 1) * 128],
                   ((C, 512),), 0, C)
            w = load_w("wk")
            proj_T(kT, lambda kc, oc, _w=w: _w[:, kc, oc * 128:(oc + 1) * 128],
                   ((256, 512), (0, 256), (768, 320)), 1, 0)
            w = load_w("wv", nc.scalar)
            proj_nat(vE, w, (0, 1, 2, 3, 4, 5, 6, 7, 8), 2)

            # ---- global-row projections + partials (overlap banded) ----
            w = load_w("wkg")
            proj_T(kgT, lambda kc, oc, _w=w: _w[:, kc, oc * 128:(oc + 1) * 128],
                   ((C, 512),), 3, C)
            w = load_w("wvg", nc.scalar)
            proj_nat(vgN, w, (2, 3, 4, 5), 4)
            w = load_w("wqg")
            # qg natural [G, H] then PE-transpose into qgT
            for o0, on in ((0, 512), (512, 256)):
                ps = psQ.tile([128, 512], F32, tag="psQ")
                for kc in range(KC):
                    nc.tensor.matmul(
                        out=ps[0:G, 0:on],
                        lhsT=hsb[:, kc, EXT:EXT + G],
                        rhs=w[:, kc, o0:o0 + on],
                        start=(kc == 0), stop=(kc == KC - 1 and not with_bias))
                if with_bias:
                    nc.tensor.matmul(
                        out=ps[0:G, 0:on], lhsT=bsb[0:1, 0:G],
                        rhs=bsb[6:7, o0:o0 + on], start=False, stop=True)
                nc.vector.tensor_copy(out=qgn[:, o0:o0 + on], in_=ps[0:G, 0:on])
            for oc in range(KC):
                pstr = psT.tile([128, 4 * VW], BF16, tag="psT")
                nc.tensor.transpose(pstr[:, 0:G],
                                    qgn[0:G, oc * 128:(oc + 1) * 128],
                                    ident[0:G, 0:G])
                nc.vector.tensor_copy(out=qgT[:, oc, :], in_=pstr[:, 0:G])

            partial = dram.tile([G, NH * VW], F32)
            reduced = dram.tile([G, NH * VW], F32)

            def glob_pair(g0):
                # two global-row heads (g0 even, g0+1) share one PSUM tile and
                # one exp; scores land transposed ([keys, 64]) so PV needs no
                # transposes. Score matmuls use disjoint PE row groups.
                g1 = g0 + 1
                # each head's scores land in a different PSUM bank so the two
                # row-group matmul streams may overlap on the PE
                pse = psQ.tile([128, 1024], F32, tag="psQ", name=f"pse{g0}")
                for kt in range(4):
                    for gi, gh in enumerate((g0, g1)):
                        dd = 64 * (gh % 2)
                        nc.tensor.matmul(
                            out=pse[:, 512 * gi + kt * G:512 * gi + (kt + 1) * G],
                            lhsT=kgT[dd:dd + 64, gh // 2,
                                     128 * kt:128 * (kt + 1)],
                            rhs=qgT[dd:dd + 64, gh // 2, :],
                            start=True, stop=True)
                exgT = work.tile([128, 512], BF16, tag="exgT", name=f"exgT{g0}")
                for gi in range(2):
                    nc.scalar.activation(out=exgT[:, 256 * gi:256 * (gi + 1)],
                                         in_=pse[:, 512 * gi:512 * gi + 256],
                                         func=Exp, scale=SCALE)
                for gi, gh in enumerate((g0, g1)):
                    ppv = psO.tile([VW, TPC], F32, tag="psO", name=f"ppv{gh}")
                    for kt in range(4):
                        nc.tensor.matmul(
                            out=ppv[0:G, 0:VW],
                            lhsT=exgT[:, 256 * gi + kt * G:256 * gi + (kt + 1) * G],
                            rhs=vgN[:, kt, VW * gh:VW * (gh + 1)],
                            start=(kt == 0), stop=(kt == 3))
                    part = w2.tile([G, VW], F32, tag="part", name=f"part{gh}")
                    nc.vector.tensor_copy(out=part, in_=ppv[0:G, 0:VW])
                    nc.sync.dma_start(out=partial[:, gh * VW:(gh + 1) * VW],
                                      in_=part)

            # ---- banded + global-column attention (the bulk) ----
            # heads processed in pairs using disjoint PE row groups; the
            # global-row pairs dovetail into the first banded pairs so the
            # AllReduce fires early and hides under the remaining banded work
            osb = late.tile([128, 4, H], BF16, tag="ph")
            for hp in range(NH // 2):
                h0, h1 = 2 * hp, 2 * hp + 1
                pc = hp
                if hp < 2:
                    glob_pair(6 * hp)
                    glob_pair(6 * hp + 2)
                    glob_pair(6 * hp + 4)
                if hp == 2:
                    nc.gpsimd.collective_compute(
                        "AllReduce", mybir.AluOpType.add,
                        replica_groups=[list(range(NCORE))],
                        ins=[partial.opt()], outs=[reduced.opt()])
                    red = late.tile([G, NH * VW], F32, tag="red")
                    nc.gpsimd.dma_start(out=red, in_=reduced)
                # -- scores for both heads, interleaved row groups --
                exs = {h0: [], h1: []}
                for pa, pb in JX_PAIRS:
                    wa = JX_T[pa][1] - JX_T[pa][0]
                    wb = JX_T[pb][1] - JX_T[pb][0]
                    pss = {}
                    for hh in (h0, h1):
                        pss[hh] = psQ.tile([128, 1024], F32, tag="psQ",
                                           name=f"pss{hh}_{pa}")
                        ex = work.tile([128, 1024], BF16, tag="ex", bufs=10)
                        exs[hh].append(ex)
                    for hh in (h0, h1):
                        dd = 64 * (hh % 2)
                        for jx, off in ((pa, 0), (pb, wa)):
                            t0, tn = JX_T[jx]
                            nc.tensor.matmul(
                                out=pss[hh][:, off:off + tn - t0],
                                lhsT=kT[dd:dd + 64, pc,
                                        128 * jx:128 * (jx + 1)],
                                rhs=qT[dd:dd + 64, pc, t0:tn],
                                start=True, stop=True)
                    for hh in (h0, h1):
                        nc.scalar.activation(out=exs[hh][-1][:, 0:wa + wb],
                                             in_=pss[hh][:, 0:wa + wb],
                                             func=Exp, scale=SCALE)
                # global-key columns for both heads
                exg = {}
                for hh in (h0, h1):
                    dd = 64 * (hh % 2)
                    pssg = psQ.tile([128, 1024], F32, tag="psQ",
                                    name=f"pssg{hh}")
                    nc.tensor.matmul(
                        out=pssg[0:G, 0:TPC],
                        lhsT=kT[dd:dd + 64, pc, EXT:EXT + G],
                        rhs=qT[dd:dd + 64, pc, :],
                        start=True, stop=True)
                    exg[hh] = work.tile([G, TPC], BF16, tag="exg",
                                        name=f"exg{hh}")
                    nc.scalar.activation(out=exg[hh], in_=pssg[0:G, 0:TPC],
                                         func=Exp, scale=SCALE)
                # -- masks --
                for hh in (h0, h1):
                    for pi, (c0, c1), (k0, k1), eng in MASK_OPS:
                        mul = nc.vector.tensor_mul if eng == "v" \
                            else nc.gpsimd.tensor_mul
                        mul(exs[hh][pi][:, c0:c1], exs[hh][pi][:, c0:c1],
                            msb[:, k0:k1])
                # -- PV + normalize per head --
                for hh in (h0, h1):
                    dd = 64 * (hh % 2)
                    pso = psO.tile([VW, TPC], F32, tag="psO")
                    first_pv = True
                    for (pa, pb), ex in zip(JX_PAIRS, exs[hh]):
                        wa = JX_T[pa][1] - JX_T[pa][0]
                        for jx, off in ((pa, 0), (pb, wa)):
                            t0, tn = JX_T[jx]
                            nc.tensor.matmul(
                                out=pso[:, t0:tn],
                                lhsT=vE[:, jx, VW * hh:VW * (hh + 1)],
                                rhs=ex[:, off:off + tn - t0],
                                start=first_pv, stop=False)
                            first_pv = False
                    nc.tensor.matmul(
                        out=pso, lhsT=vE[0:G, 8, VW * hh:VW * (hh + 1)],
                        rhs=exg[hh], start=False, stop=True)
                    ot = w2.tile([VW, TPC], BF16, tag="ot")
                    nc.vector.tensor_copy(out=ot, in_=pso)
                    # 4 transposes into one psum tile, merged reciprocal
                    pstr = psT.tile([128, 4 * VW], BF16, tag="psT")
                    for tt in range(4):
                        nc.tensor.transpose(pstr[:, tt * VW:(tt + 1) * VW],
                                            ot[:, tt * 128:(tt + 1) * 128],
                                            ident[0:VW, 0:VW])
                    rec = work.tile([128, 4], F32, tag="rec", bufs=4)
                    nc.vector.reciprocal(
                        out=rec,
                        in_=pstr.rearrange("p (tt x) -> p tt x", x=VW)[:, :, 64])
                    for tt in range(4):
                        nc.vector.tensor_scalar_mul(
                            osb[:, tt, 64 * hh:64 * (hh + 1)],
                            in0=pstr[:, tt * VW:tt * VW + 64],
                            scalar1=rec[:, tt:tt + 1])
            for i4 in range(4):
                eng = nc.sync if i4 % 2 == 0 else nc.scalar
                eng.dma_start(out=d_out[128 * i4:128 * (i4 + 1), :],
                              in_=osb[:, i4, :])

            # ---- finalize global rows from the AllReduced partials ----
            ogsb = late.tile([G, H], F32, tag="ogsb")
            for h in range(NH):
                recg = work.tile([G, 1], F32, tag="recg")
                nc.vector.reciprocal(out=recg,
                                     in_=red[:, h * VW + 64:h * VW + 65])
                nc.vector.tensor_scalar_mul(
                    ogsb[:, h * 64:(h + 1) * 64],
                    in0=red[:, h * VW:h * VW + 64], scalar1=recg)
            nc.sync.dma_start(out=d_outg[:], in_=ogsb)

    nc.compile()
    return nc


def _host_inputs(hs, weights, biases):
    """Build the 8 per-core input maps from full inputs."""
    BF = ml_dtypes.bfloat16
    hsT = np.ascontiguousarray(hs.T).astype(BF)    # [H, S]
    weights_bf = [w.astype(BF) for w in weights]
    consts = np.eye(128, dtype=BF)

    with_bias = any(np.any(b) for b in biases)
    if with_bias:
        brow = np.zeros((7, COLS), BF)
        brow[0, :] = 1.0
        for i, b in enumerate(biases):
            brow[1 + i, :H] = b.astype(BF)
    pp = np.arange(128)[:, None]                    # key pos within jx tile
    ii = np.arange(C)[None, :]                      # query pos within chunk
    in_maps = []
    for core in range(NCORE):
        hst = np.zeros((H, COLS), BF)
        lo = TPC * core - C
        hi = TPC * core + TPC + C
        clo, chi = max(lo, 0), min(hi, S)
        hst[:, clo - lo:chi - lo] = hsT[:, clo:chi]
        hst[:, EXT:EXT + G] = hsT[:, :G]
        hst[:, EXT + G:] = hsT[:, :G]               # duplicate global tokens
        # masks packed per jx at JX_PACK offsets, covering JX_MASK regions
        mk = np.ones((128, 2304), BF)
        for jx in range(8):
            m0, m1 = JX_MASK[jx]
            mo = JX_PACK[jx]
            for cl in range(2):
                jt = jx - 2 * cl
                if not 0 <= jt <= 5:
                    continue
                t_lo = C * cl
                if t_lo < m0 or t_lo >= m1:
                    continue
                n = 2 * core + cl
                jj = 128 * jt + pp                  # strip pos within chunk
                ka = n * C - C + jj                 # absolute key pos
                valid = ((jj >= ii) & (jj <= ii + 2 * C)
                         & (ka >= G) & (ka < S))
                mk[:, mo + t_lo - m0:mo + t_lo - m0 + C] = valid
        im = {
            "hsT": hst,
            "wq": weights_bf[0], "wk": weights_bf[1], "wv": weights_bf[2],
            "wkg": weights_bf[3], "wvg": weights_bf[4], "wqg": weights_bf[5],
            "masks": mk,
            "consts": consts,
        }
        if with_bias:
            im["biasrow"] = brow
        in_maps.append(im)
    return in_maps, with_bias


def kernel(hidden_states, Wq, bq, Wk, bk, Wv, bv, Wqg, bqg, Wkg, bkg,
           Wvg, bvg):
    hs = np.asarray(hidden_states, np.float32).reshape(S, H)
    weights = [np.ascontiguousarray(np.asarray(w, np.float32))
               for w in (Wq, Wk, Wv, Wkg, Wvg, Wqg)]
    biases = [np.asarray(b, np.float32)
              for b in (bq, bk, bv, bkg, bvg, bqg)]
    in_maps, with_bias = _host_inputs(hs, weights, biases)

    if with_bias not in _PROG_CACHE:
        _PROG_CACHE[with_bias] = _build_program(with_bias)
    nc = _PROG_CACHE[with_bias]

    res = run_bass_kernel_spmd(nc, in_maps, list(range(NCORE)))

    out = np.empty((S, H), np.float32)
    for core in range(NCORE):
        out[TPC * core:TPC * (core + 1)] = np.asarray(
            res.results[core]["out"], np.float32)
    out[:G] = res.results[0]["outg"]
    return out.reshape(1, S, H)


# revision 45
# speedup vs baseline: 1.7174x; 1.7174x over previous
"""Longformer sliding-window + global attention layer on 8 Trainium2 NeuronCores.

Sharding: sequence-parallel over the 4096 tokens (512 per core, all 12 heads).
Each core recomputes the k/v halo (256 tokens each side) and the 64 global
k/v tokens locally from zero-padded hsT input, so the program is uniform SPMD.
The global-query rows (first 64 tokens attend to everything) are computed as
flash-style partial sums over each core's 512 tokens and combined with an
on-device AllReduce (hidden under the banded phase); every core finalizes the
identical 64 global rows.

Layout strategy (all matmuls bf16, accumulation fp32 PSUM):
  - hsT [hidden, tokens] feeds projections in both orientations. The 64
    global tokens appear twice at the tail (cols 1024:1088 and 1088:1152) so
    the natural-v projection materializes global v rows in both partition
    halves, letting the two heads of a pair use disjoint PE row groups.
  - Banded attention processes heads in pairs (head dims at partitions 0:64
    and 64:128): their K=64 score matmuls target disjoint PE row groups and
    run concurrently. Scores are transposed ([keys, queries]) over 8 extended
    window key tiles (jx) in pairs sharing a 2-bank PSUM tile so one exp
    covers up to 1024 columns. 0/1 mask multiplies cover only band/col
    boundary regions. PV accumulates into a [66, 512] PSUM tile via natural-v
    tiles carrying an appended ones-column, so the softmax denominator falls
    out of the same accumulation. A PE transpose + per-partition reciprocal
    scale normalizes [t, d] tiles.
  - Global-row heads compute scores directly transposed ([keys, 64]) via
    four 128-key strip matmuls per head, two heads sharing one PSUM tile and
    one 512-col exp, then PV with the exp strips as stationary operands -
    no PE transposes.
"""
import numpy as np
import ml_dtypes

import concourse.bacc as bacc
import concourse.mybir as mybir
import concourse.tile as tile
from concourse.bass_utils import run_bass_kernel_spmd

F32 = mybir.dt.float32
BF16 = mybir.dt.bfloat16
Exp = mybir.ActivationFunctionType.Exp

S, H, NH, HD = 4096, 768, 12, 64
C = 256               # chunk / one-sided window
G = 64                # global tokens
NCORE = 8
TPC = S // NCORE      # 512 tokens per core
EXT = TPC + 2 * C     # 1024 ext window
COLS = EXT + 2 * G    # 1152 = ext | glob | glob-dup
KC = H // 128         # 6 hidden chunks
VW = 66               # per-head v block: 64 v | ones | pad
SCALE = 1.0 / 8.0     # 1/sqrt(HD)

# per key-tile jx: (t0, tn) query range its chunk windows cover, and the
# t-range needing the 0/1 mask multiply
JX_T = {0: (0, 256), 1: (0, 256), 2: (0, 512), 3: (0, 512),
        4: (0, 512), 5: (0, 512), 6: (256, 512), 7: (256, 512)}
JX_MASK = {0: (0, 256), 1: (0, 256), 2: (0, 512), 3: (256, 512),
           4: (0, 256), 5: (0, 256), 6: (256, 512), 7: (256, 512)}
# pairs share one [128, 1024] PSUM tile; (2,3) first so PV start=True is full
JX_PAIRS = [(2, 3), (4, 5), (0, 1), (6, 7)]
# packed col offset of each jx's mask region in the [128, 2304] masks input
JX_PACK = {2: 0, 3: 512, 4: 768, 5: 1024, 0: 1280, 1: 1536, 6: 1792, 7: 2048}
# mask ops: (pair_index, ex-tile col range, packed range, engine)
MASK_OPS = [
    (0, (0, 512), (0, 512), "v"),         # jx2
    (0, (768, 1024), (512, 768), "v"),    # jx3
    (1, (0, 256), (768, 1024), "v"),      # jx4
    (1, (512, 768), (1024, 1280), "v"),   # jx5
    (2, (0, 512), (1280, 1792), "v"),     # jx0|jx1 contiguous
    (3, (0, 512), (1792, 2304), "v"),     # jx6|jx7 contiguous
]

_PROG_CACHE = {}


def _build_program(with_bias: bool):
    nc = bacc.Bacc("TRN2", target_bir_lowering=False, debug=False,
                   num_devices=NCORE)
    d_hsT = nc.declare_dram_parameter("hsT", [H, COLS], BF16, isOutput=False)
    d_w = {
        n: nc.declare_dram_parameter(n, [H, H], BF16, isOutput=False)
        for n in ("wq", "wk", "wv", "wkg", "wvg", "wqg")
    }
    d_masks = nc.declare_dram_parameter("masks", [128, 2304], BF16,
                                        isOutput=False)
    d_consts = nc.declare_dram_parameter("consts", [128, 128], BF16,
                                         isOutput=False)
    if with_bias:
        d_brow = nc.declare_dram_parameter("biasrow", [7, COLS], BF16,
                                           isOutput=False)
    d_out = nc.declare_dram_parameter("out", [TPC, H], BF16, isOutput=True)
    d_outg = nc.declare_dram_parameter("outg", [G, H], F32, isOutput=True)

    with tile.TileContext(nc) as tc:
        with (
            tc.tile_pool(name="const", bufs=1) as const,
            tc.tile_pool(name="wfull", bufs=2) as wfull,
            tc.tile_pool(name="work", bufs=2) as work,
            tc.tile_pool(name="w2", bufs=3) as w2,
            tc.tile_pool(name="late", bufs=1) as late,
            tc.tile_pool(name="dram", bufs=2, space="DRAM") as dram,
            tc.tile_pool(name="psQ", bufs=2, space="PSUM") as psQ,
            tc.tile_pool(name="psO", bufs=2, space="PSUM") as psO,
            tc.tile_pool(name="psT", bufs=2, space="PSUM") as psT,
        ):
            # ---- resident loads; spread the startup DMAs across queues ----
            hsb = late.tile([128, KC, COLS], BF16, tag="ph")
            wq_t = wfull.tile([128, KC, H], BF16, tag="wq")
            for kc in range(KC):
                nc.sync.dma_start(
                    out=wq_t[:, kc, :],
                    in_=d_w["wq"][128 * kc:128 * (kc + 1), :])
                eng = nc.gpsimd if kc % 2 == 0 else nc.scalar
                eng.dma_start(
                    out=hsb[:, kc, 256:768],
                    in_=d_hsT[128 * kc:128 * (kc + 1), 256:768])
            for kc in range(KC):
                eng = nc.gpsimd if kc % 2 == 0 else nc.scalar
                eng.dma_start(
                    out=hsb[:, kc, 0:256],
                    in_=d_hsT[128 * kc:128 * (kc + 1), 0:256])
                eng.dma_start(
                    out=hsb[:, kc, 768:COLS],
                    in_=d_hsT[128 * kc:128 * (kc + 1), 768:COLS])

            def load_w(name, eng=None):
                t = wfull.tile([128, KC, H], BF16, tag="wfull")
                (eng or nc.sync).dma_start(
                    out=t, in_=d_w[name].rearrange("(kc p) o -> p kc o", p=128))
                return t

            csb = const.tile([128, 128], BF16)
            nc.gpsimd.dma_start(out=csb, in_=d_consts[:])
            ident = csb
            if with_bias:
                bsb = const.tile([7, COLS], BF16)
                nc.gpsimd.dma_start(out=bsb, in_=d_brow[:])

            KCOLS = EXT + G                           # kT covers one glob copy
            kT = const.tile([128, KC, KCOLS], BF16)   # [o, t] all heads
            qT = const.tile([128, KC, TPC], BF16)
            vE = const.tile([128, 9, NH * VW], BF16)  # natural v + ones cols
            kgT = const.tile([128, KC, TPC], BF16)
            vgN = const.tile([128, 4, NH * VW], BF16)
            qgT = const.tile([128, KC, G], BF16)
            qgn = const.tile([G, H], BF16)
            msb = const.tile([128, 2304], BF16)
            nc.gpsimd.dma_start(out=msb, in_=d_masks[:])
            # ones/pad columns of the natural-v blocks (cols 64,65 of each
            # 66-block); value cols are overwritten by the projections
            nc.gpsimd.memset(
                vE.rearrange("p a (h x) -> p a h x", x=VW)[:, :, :, 64:66], 1.0)
            nc.gpsimd.memset(
                vgN.rearrange("p a (h x) -> p a h x", x=VW)[:, :, :, 64:66], 1.0)

            def proj_T(dst, wsl, segs, bias_idx, dst_off):
                # dst[o, t] = W.T @ hsT cols; wsl(kc, oc) -> [128, 128] strip
                for oc in range(KC):
                    for c0, cn in segs:
                        ps = psQ.tile([128, 512], F32, tag="psQ")
                        for kc in range(KC):
                            nc.tensor.matmul(
                                out=ps[:, 0:cn],
                                lhsT=wsl(kc, oc),
                                rhs=hsb[:, kc, c0:c0 + cn],
                                start=(kc == 0),
                                stop=(kc == KC - 1 and not with_bias),
                            )
                        if with_bias:
                            nc.tensor.matmul(
                                out=ps[:, 0:cn],
                                lhsT=bsb[1 + bias_idx:2 + bias_idx,
                                         oc * 128:(oc + 1) * 128],
                                rhs=bsb[0:1, 0:cn],
                                start=False, stop=True,
                            )
                        nc.vector.tensor_copy(
                            out=dst[:, oc, c0 - dst_off:c0 - dst_off + cn],
                            in_=ps[:, 0:cn])

            def proj_nat(dst, wsb, tts, bias_idx):
                # dst[t, head-block] with 66-stride head blocks
                for ti, tt in enumerate(tts):
                    tok0 = tt * 128
                    for o0, on in ((0, 512), (512, 256)):
                        ps = psQ.tile([128, 512], F32, tag="psQ")
                        for kc in range(KC):
                            nc.tensor.matmul(
                                out=ps[:, 0:on],
                                lhsT=hsb[:, kc, tok0:tok0 + 128],
                                rhs=wsb[:, kc, o0:o0 + on],
                                start=(kc == 0),
                                stop=(kc == KC - 1 and not with_bias),
                            )
                        if with_bias:
                            nc.tensor.matmul(
                                out=ps[:, 0:on],
                                lhsT=bsb[0:1, 0:128],
                                rhs=bsb[1 + bias_idx:2 + bias_idx, o0:o0 + on],
                                start=False, stop=True,
                            )
                        nc.vector.tensor_copy(
                            out=dst[:, ti, :].rearrange(
                                "p (h x) -> p h x", x=VW)[:, o0 // 64:(o0 + on) // 64, 0:64],
                            in_=ps[:, 0:on].rearrange("p (h x) -> p h x", x=64))

            # ---- main projections first (banded inputs ready earliest) ----
            proj_T(qT, lambda kc, oc: wq_t[:, kc, oc * 128:(oc + 1) * 128],
                   ((C, 512),), 0, C)
            w = load_w("wk")
            proj_T(kT, lambda kc, oc, _w=w: _w[:, kc, oc * 128:(oc + 1) * 128],
                   ((256, 512), (0, 256), (768, 320)), 1, 0)
            w = load_w("wv", nc.scalar)
            proj_nat(vE, w, (0, 1, 2, 3, 4, 5, 6, 7, 8), 2)

            # ---- global-row projections + partials (overlap banded) ----
            w = load_w("wkg")
            proj_T(kgT, lambda kc, oc, _w=w: _w[:, kc, oc * 128:(oc + 1) * 128],
                   ((C, 512),), 3, C)
            w = load_w("wvg", nc.scalar)
            proj_nat(vgN, w, (2, 3, 4, 5), 4)
            w = load_w("wqg")
            # qg natural [G, H] then PE-transpose into qgT
            for o0, on in ((0, 512), (512, 256)):
                ps = psQ.tile([128, 512], F32, tag="psQ")
                for kc in range(KC):
                    nc.tensor.matmul(
                        out=ps[0:G, 0:on],
                        lhsT=hsb[:, kc, EXT:EXT + G],
                        rhs=w[:, kc, o0:o0 + on],
                        start=(kc == 0), stop=(kc == KC - 1 and not with_bias))
                if with_bias:
                    nc.tensor.matmul(
                        out=ps[0:G, 0:on], lhsT=bsb[0:1, 0:G],
                        rhs=bsb[6:7, o0:o0 + on], start=False, stop=True)
                nc.vector.tensor_copy(out=qgn[:, o0:o0 + on], in_=ps[0:G, 0:on])
            for oc in range(KC):
                pstr = psT.tile([128, 4 * VW], BF16, tag="psT")
                nc.tensor.transpose(pstr[:, 0:G],
                                    qgn[0:G, oc * 128:(oc + 1) * 128],
                                    ident[0:G, 0:G])
                nc.vector.tensor_copy(out=qgT[:, oc, :], in_=pstr[:, 0:G])

            partial = dram.tile([G, NH * VW], F32)
            reduced = dram.tile([G, NH * VW], F32)

            def glob_pair(g0):
                # two global-row heads (g0 even, g0+1) share one PSUM tile and
                # one exp; scores land transposed ([keys, 64]) so PV needs no
                # transposes. Score matmuls use disjoint PE row groups.
                g1 = g0 + 1
                # each head's scores land in a different PSUM bank so the two
                # row-group matmul streams may overlap on the PE
                pse = psQ.tile([128, 1024], F32, tag="psQ", name=f"pse{g0}")
                for kt in range(4):
                    for gi, gh in enumerate((g0, g1)):
                        dd = 64 * (gh % 2)
                        nc.tensor.matmul(
                            out=pse[:, 512 * gi + kt * G:512 * gi + (kt + 1) * G],
                            lhsT=kgT[dd:dd + 64, gh // 2,
                                     128 * kt:128 * (kt + 1)],
                            rhs=qgT[dd:dd + 64, gh // 2, :],
                            start=True, stop=True)
                exgT = work.tile([128, 512], BF16, tag="exgT", name=f"exgT{g0}")
                for gi in range(2):
                    nc.scalar.activation(out=exgT[:, 256 * gi:256 * (gi + 1)],
                                         in_=pse[:, 512 * gi:512 * gi + 256],
                                         func=Exp, scale=SCALE)
                for gi, gh in enumerate((g0, g1)):
                    ppv = psO.tile([VW, TPC], F32, tag="psO", name=f"ppv{gh}")
                    for kt in range(4):
                        nc.tensor.matmul(
                            out=ppv[0:G, 0:VW],
                            lhsT=exgT[:, 256 * gi + kt * G:256 * gi + (kt + 1) * G],
                            rhs=vgN[:, kt, VW * gh:VW * (gh + 1)],
                            start=(kt == 0), stop=(kt == 3))
                    part = w2.tile([G, VW], F32, tag="part", name=f"part{gh}")
                    nc.vector.tensor_copy(out=part, in_=ppv[0:G, 0:VW])
                    nc.sync.dma_start(out=partial[:, gh * VW:(gh + 1) * VW],
                                      in_=part)

            # ---- banded + global-column attention (the bulk) ----
            # heads processed in pairs using disjoint PE row groups; the
            # global-row pairs dovetail into the first banded pairs so the
            # AllReduce fires early and hides under the remaining banded work
            osb = late.tile([128, 4, H], BF16, tag="ph")
            for hp in range(NH // 2):
                h0, h1 = 2 * hp, 2 * hp + 1
                pc = hp
                if hp < 2:
                    glob_pair(6 * hp)
                    glob_pair(6 * hp + 2)
                    glob_pair(6 * hp + 4)
                if hp == 2:
                    nc.gpsimd.collective_compute(
                        "AllReduce", mybir.AluOpType.add,
                        replica_groups=[list(range(NCORE))],
                        ins=[partial.opt()], outs=[reduced.opt()])
                    red = late.tile([G, NH * VW], F32, tag="red")
                    nc.gpsimd.dma_start(out=red, in_=reduced)
                # -- scores for both heads, interleaved row groups --
                exs = {h0: [], h1: []}
                for pa, pb in JX_PAIRS:
                    wa = JX_T[pa][1] - JX_T[pa][0]
                    wb = JX_T[pb][1] - JX_T[pb][0]
                    pss = {}
                    for hh in (h0, h1):
                        pss[hh] = psQ.tile([128, 1024], F32, tag="psQ",
                                           name=f"pss{hh}_{pa}")
                        ex = work.tile([128, 1024], BF16, tag="ex", bufs=10)
                        exs[hh].append(ex)
                    for hh in (h0, h1):
                        dd = 64 * (hh % 2)
                        for jx, off in ((pa, 0), (pb, wa)):
                            t0, tn = JX_T[jx]
                            nc.tensor.matmul(
                                out=pss[hh][:, off:off + tn - t0],
                                lhsT=kT[dd:dd + 64, pc,
                                        128 * jx:128 * (jx + 1)],
                                rhs=qT[dd:dd + 64, pc, t0:tn],
                                start=True, stop=True)
                    for hh in (h0, h1):
                        nc.scalar.activation(out=exs[hh][-1][:, 0:wa + wb],
                                             in_=pss[hh][:, 0:wa + wb],
                                             func=Exp, scale=SCALE)
                # global-key columns for both heads
                exg = {}
                for hh in (h0, h1):
                    dd = 64 * (hh % 2)
                    pssg = psQ.tile([128, 1024], F32, tag="psQ",
                                    name=f"pssg{hh}")
                    nc.tensor.matmul(
                        out=pssg[0:G, 0:TPC],
                        lhsT=kT[dd:dd + 64, pc, EXT:EXT + G],
                        rhs=qT[dd:dd + 64, pc, :],
                        start=True, stop=True)
                    exg[hh] = work.tile([G, TPC], BF16, tag="exg",
                                        name=f"exg{hh}")
                    nc.scalar.activation(out=exg[hh], in_=pssg[0:G, 0:TPC],
                                         func=Exp, scale=SCALE)
                # -- masks --
                for hh in (h0, h1):
                    for pi, (c0, c1), (k0, k1), eng in MASK_OPS:
                        mul = nc.vector.tensor_mul if eng == "v" \
                            else nc.gpsimd.tensor_mul
                        mul(exs[hh][pi][:, c0:c1], exs[hh][pi][:, c0:c1],
                            msb[:, k0:k1])
                # -- PV + normalize per head --
                for hh in (h0, h1):
                    dd = 64 * (hh % 2)
                    pso = psO.tile([VW, TPC], F32, tag="psO")
                    first_pv = True
                    for (pa, pb), ex in zip(JX_PAIRS, exs[hh]):
                        wa = JX_T[pa][1] - JX_T[pa][0]
                        for jx, off in ((pa, 0), (pb, wa)):
                            t0, tn = JX_T[jx]
                            nc.tensor.matmul(
                                out=pso[:, t0:tn],
                                lhsT=vE[:, jx, VW * hh:VW * (hh + 1)],
                                rhs=ex[:, off:off + tn - t0],
                                start=first_pv, stop=False)
                            first_pv = False
                    nc.tensor.matmul(
                        out=pso, lhsT=vE[0:G, 8, VW * hh:VW * (hh + 1)],
                        rhs=exg[hh], start=False, stop=True)
                    ot = w2.tile([VW, TPC], BF16, tag="ot")
                    nc.vector.tensor_copy(out=ot, in_=pso)
                    # 4 transposes into one psum tile, merged reciprocal
                    pstr = psT.tile([128, 4 * VW], BF16, tag="psT")
                    for tt in range(4):
                        nc.tensor.transpose(pstr[:, tt * VW:(tt + 1) * VW],
                                            ot[:, tt * 128:(tt + 1) * 128],
                                            ident[0:VW, 0:VW])
                    rec = work.tile([128, 4], F32, tag="rec", bufs=4)
                    nc.vector.reciprocal(
                        out=rec,
                        in_=pstr.rearrange("p (tt x) -> p tt x", x=VW)[:, :, 64])
                    for tt in range(4):
                        nc.vector.tensor_scalar_mul(
                            osb[:, tt, 64 * hh:64 * (hh + 1)],
                            in0=pstr[:, tt * VW:tt * VW + 64],
                            scalar1=rec[:, tt:tt + 1])
            for i4 in range(4):
                eng = nc.sync if i4 % 2 == 0 else nc.scalar
                eng.dma_start(out=d_out[128 * i4:128 * (i4 + 1), :],
                              in_=osb[:, i4, :])

            # ---- finalize global rows from the AllReduced partials ----
            ogsb = late.tile([G, H], F32, tag="ogsb")
            rec12 = work.tile([G, NH], F32, tag="recg")
            nc.vector.reciprocal(
                out=rec12,
                in_=red.rearrange("p (h x) -> p h x", x=VW)[:, :, 64])
            for h in range(NH):
                eng = nc.vector if h % 2 == 0 else nc.gpsimd
                eng.tensor_scalar_mul(
                    ogsb[:, h * 64:(h + 1) * 64],
                    in0=red[:, h * VW:h * VW + 64],
                    scalar1=rec12[:, h:h + 1])
            nc.sync.dma_start(out=d_outg[:], in_=ogsb)

    nc.compile()
    return nc


def _host_inputs(hs, weights, biases):
    """Build the 8 per-core input maps from full inputs."""
    BF = ml_dtypes.bfloat16
    hsT = np.ascontiguousarray(hs.T).astype(BF)    # [H, S]
    weights_bf = [w.astype(BF) for w in weights]
    consts = np.eye(128, dtype=BF)

    with_bias = any(np.any(b) for b in biases)
    if with_bias:
        brow = np.zeros((7, COLS), BF)
        brow[0, :] = 1.0
        for i, b in enumerate(biases):
            brow[1 + i, :H] = b.astype(BF)
    pp = np.arange(128)[:, None]                    # key pos within jx tile
    ii = np.arange(C)[None, :]                      # query pos within chunk
    in_maps = []
    for core in range(NCORE):
        hst = np.zeros((H, COLS), BF)
        lo = TPC * core - C
        hi = TPC * core + TPC + C
        clo, chi = max(lo, 0), min(hi, S)
        hst[:, clo - lo:chi - lo] = hsT[:, clo:chi]
        hst[:, EXT:EXT + G] = hsT[:, :G]
        hst[:, EXT + G:] = hsT[:, :G]               # duplicate global tokens
        # masks packed per jx at JX_PACK offsets, covering JX_MASK regions
        mk = np.ones((128, 2304), BF)
        for jx in range(8):
            m0, m1 = JX_MASK[jx]
            mo = JX_PACK[jx]
            for cl in range(2):
                jt = jx - 2 * cl
                if not 0 <= jt <= 5:
                    continue
                t_lo = C * cl
                if t_lo < m0 or t_lo >= m1:
                    continue
                n = 2 * core + cl
                jj = 128 * jt + pp                  # strip pos within chunk
                ka = n * C - C + jj                 # absolute key pos
                valid = ((jj >= ii) & (jj <= ii + 2 * C)
                         & (ka >= G) & (ka < S))
                mk[:, mo + t_lo - m0:mo + t_lo - m0 + C] = valid
        im = {
            "hsT": hst,
            "wq": weights_bf[0], "wk": weights_bf[1], "wv": weights_bf[2],
            "wkg": weights_bf[3], "wvg": weights_bf[4], "wqg": weights_bf[5],
            "masks": mk,
            "consts": consts,
        }
        if with_bias:
            im["biasrow"] = brow
        in_maps.append(im)
    return in_maps, with_bias


def kernel(hidden_states, Wq, bq, Wk, bk, Wv, bv, Wqg, bqg, Wkg, bkg,
           Wvg, bvg):
    hs = np.asarray(hidden_states, np.float32).reshape(S, H)
    weights = [np.ascontiguousarray(np.asarray(w, np.float32))
               for w in (Wq, Wk, Wv, Wkg, Wvg, Wqg)]
    biases = [np.asarray(b, np.float32)
              for b in (bq, bk, bv, bkg, bvg, bqg)]
    in_maps, with_bias = _host_inputs(hs, weights, biases)

    if with_bias not in _PROG_CACHE:
        _PROG_CACHE[with_bias] = _build_program(with_bias)
    nc = _PROG_CACHE[with_bias]

    res = run_bass_kernel_spmd(nc, in_maps, list(range(NCORE)))

    out = np.empty((S, H), np.float32)
    for core in range(NCORE):
        out[TPC * core:TPC * (core + 1)] = np.asarray(
            res.results[core]["out"], np.float32)
    out[:G] = res.results[0]["outg"]
    return out.reshape(1, S, H)
